# revision 70
# baseline (speedup 1.0000x reference)
"""Trainium2 Bass kernel for GRU seq2seq w/ Bahdanau attention (nn_DSkBart).

Sharding (8 NeuronCores):
  * recurrence (bi-GRU encoder + attention decoder): data-parallel over
    batch, 8 batch cols per core, single fused chain per core in
    transposed-state layout ([feature-on-partitions, batch-on-free]).
  * fc_out: 2D-sharded (vocab x batch-half): core c holds fc_W^T cols
    [(c%4)*8000, ...) resident in SBUF (bf16) and computes logits for
    batch half c//4; decoder states are AllGather'd within each 4-core
    group (halving collective bytes vs world gathers) in staged chunks
    that interleave the big fc matmul with the recurrence on the PE.

Latency/throughput tricks (the kernel is bound by the decoder's serial
dependency chain, with the fc matmul filling PE gaps):
  * sigmoid(x)=(tanh(x/2)+1)/2 keeps one ACT table set; Whh/Wihw n-gate
    rows host-prescaled by 0.5 so r*g_n is one fused DVE op.
  * embedding gate contributions fold into the gate matmuls as extra
    K-tiles ([h; emb] for the encoder, [w; emb] for the decoder) -- no
    separate Wih@emb pass, no PSUM preloads (PSUM preload + accumulate
    proved numerically unsafe on HW).
  * energy adds use per-(kt,b) tensor_scalar with q as the per-partition
    scalar (packed bf16 operands -> fast DVE mode), tanh in 4 quarters
    so score matmuls start early.
  * softmax denominators replicated across partitions via a block-ones
    matmul; all small weights arrive in one packed blob DMA; fc weights
    stream in 7 chunks under the encoder; logits leave in bf16 (host
    converts to f32); collectives are the only Pool-engine work during
    decode (a collective holds its sequencer for the whole transfer,
    and GPSIMD cannot touch PSUM).

Self-contained: hardcodes all shapes; host does layout/dtype prep only.
"""

import numpy as np
import ml_dtypes

import concourse.bass as bass
import concourse.bacc as bacc_mod
import concourse.tile as tile
import concourse.mybir as mybir
from concourse.bass_utils import run_bass_kernel_spmd

# problem dims
V, S, T, B = 32000, 64, 64, 64
E, EH, DH = 128, 256, 256
NCORES = 8
NGRP = 4                  # cores per batch-half group
BL = B // NCORES          # 8 batch cols per core
HB = NGRP * BL            # 32 batch cols per group (batch half)
TD = T - 1                # 63 decoder steps
M = TD * HB               # 2016 output rows per core
KX = DH + 2 * EH + E      # 896 = xcat dim
KT_X = KX // 128          # 7
VS = V // NGRP            # 8000 vocab cols per core
NSUB = 250                # fc psum n-subtile
NCH = VS // NSUB          # 16 n-chunks

F32 = mybir.dt.float32
BF16 = mybir.dt.bfloat16
I32 = mybir.dt.int32
AF = mybir.ActivationFunctionType
OP = mybir.AluOpType
bfnp = ml_dtypes.bfloat16

# (t_issue, t0_start, n_steps); sum of n_steps == 63
GATHERS = ((3, 0, 4), (7, 4, 4), (15, 8, 8), (23, 16, 8), (31, 24, 8),
           (39, 32, 8), (47, 40, 8), (51, 48, 4), (55, 52, 4), (59, 56, 4),
           (62, 60, 3))

_CACHE = {}


def _build_program():
    nc = bacc_mod.Bacc("TRN2", num_devices=NCORES)

    # ---- DRAM I/O ----
    tok_idx_d = nc.dram_tensor("tok_idx", [128, 8], I32, kind="ExternalInput")
    enc_emb_d = nc.dram_tensor("enc_emb", [V, E], BF16, kind="ExternalInput")
    dec_emb_d = nc.dram_tensor("dec_emb", [V, E], BF16, kind="ExternalInput")
    wihf_d = nc.dram_tensor("wihf_t", [E, 3 * EH], BF16, kind="ExternalInput")
    wihb_d = nc.dram_tensor("wihb_t", [E, 3 * EH], BF16, kind="ExternalInput")
    whhf_d = nc.dram_tensor("whhf_t", [EH, 3 * EH], BF16, kind="ExternalInput")
    whhb_d = nc.dram_tensor("whhb_t", [EH, 3 * EH], BF16, kind="ExternalInput")
    encfc_d = nc.dram_tensor("encfc_t", [2 * EH, DH], BF16, kind="ExternalInput")
    wh_d = nc.dram_tensor("wh_t", [DH, DH], BF16, kind="ExternalInput")
    we_d = nc.dram_tensor("we_t", [2 * EH, DH], BF16, kind="ExternalInput")
    v_d = nc.dram_tensor("v_att", [128, 2], BF16, kind="ExternalInput")
    wihe_d = nc.dram_tensor("wihe_t", [E, 3 * DH], BF16, kind="ExternalInput")
    wihw_d = nc.dram_tensor("wihw_t", [2 * EH, 3 * DH], BF16, kind="ExternalInput")
    whhd_d = nc.dram_tensor("whhd_t", [DH, 3 * DH], BF16, kind="ExternalInput")
    fcwt_d = nc.dram_tensor("fcw_t", [KX, VS], BF16, kind="ExternalInput")
    xg_in = [nc.dram_tensor(f"xg_in{g}", [128, 7, BL * ns], BF16)
             for g, (_, _, ns) in enumerate(GATHERS)]
    xg_out = [nc.dram_tensor(f"xg_out{g}", [NGRP, 128, 7, BL * ns], BF16)
              for g, (_, _, ns) in enumerate(GATHERS)]
    blk_d = nc.dram_tensor("blkones", [128, 128], F32, kind="ExternalInput")
    out_d = nc.dram_tensor("out", [M, VS], BF16, kind="ExternalOutput")

    with tile.TileContext(nc) as tc, \
            tc.tile_pool(name="singles", bufs=1) as sing, \
            tc.tile_pool(name="steps", bufs=3) as stp, \
            tc.tile_pool(name="fcout", bufs=4) as fco_pool:

        def kload(dram, ksub, mdim):
            t = sing.tile([128, ksub, mdim], BF16, tag=dram.name)
            nc.sync.dma_start(t[:], dram[:].rearrange("(ko p) m -> p ko m", p=128))
            return t

        whhf = kload(whhf_d, 2, 768)
        whhb = kload(whhb_d, 2, 768)
        encfc = kload(encfc_d, 4, 256)
        wh = kload(wh_d, 2, 256)
        we = kload(we_d, 4, 256)
        wihw = kload(wihw_d, 4, 768)
        whhd = kload(whhd_d, 2, 768)
        v_sb = sing.tile([128, 2], BF16, tag="v_sb")
        nc.sync.dma_start(v_sb[:], v_d[:])
        ident = sing.tile([128, 128], BF16, tag="ident")
        nc.sync.dma_start(ident[:], ident_d[:])
        blk = sing.tile([128, 128], F32, tag="blk")
        nc.sync.dma_start(blk[:], blk_d[:])
        fcw_sb = sing.tile([128, 7, VS], BF16, tag="fcw_sb")

        # persistent activations
        embT_enc = sing.tile([128, 4, 128], BF16, tag="embT_enc")   # [E,(s,b)]
        embT_dec = sing.tile([128, 4, 128], BF16, tag="embT_dec")   # [E,(t,b)]
        encT = sing.tile([128, 4, 512], BF16, tag="encT")           # [2EH,(b,s)]
        enc_pack = sing.tile([128, 4, 512], BF16, tag="enc_pack")   # [(b%2,s),(b//2,e)]
        enc_proj = sing.tile([128, 2, 512], BF16, tag="enc_proj")   # [DH,(b,s)]
        xcatT = sing.tile([128, 7, 512], BF16, tag="xcatT")         # [896,(t,b)]
        h_enc = sing.tile([128, 2, 16], BF16, tag="h_enc")          # enc f|b state
        h0_bf = sing.tile([128, 2, BL], BF16, tag="h0_bf")
        a_eo = sing.tile([128, 4, 2], BF16, tag="a_eo")

        nc.vector.memset(h_enc[:], 0.0)
        nc.vector.memset(xcatT[:, :, 504:512], 0.0)
        nc.vector.memset(a_eo[:], 0.0)

        # ---------- setup: embedding gathers + transposes ----------
        with tc.tile_pool(name="setup", bufs=4) as setp, \
                tc.tile_pool(name="psetup", bufs=2, space="PSUM") as psetp:
            idx_all = sing.tile([128, 8], I32, tag="idx_all")
            nc.sync.dma_start(idx_all[:], tok_idx_d[:])
            for ti, (table, dstT) in enumerate(((enc_emb_d, embT_enc),
                                                (dec_emb_d, embT_dec))):
                for g in range(4):
                    emb_g = setp.tile([128, 128], BF16, tag="embg")
                    nc.gpsimd.indirect_dma_start(
                        out=emb_g[:], out_offset=None, in_=table[:],
                        in_offset=bass.IndirectOffsetOnAxis(
                            ap=idx_all[:, ti * 4 + g:ti * 4 + g + 1], axis=0))
                    pt = psetp.tile([128, 128], BF16, tag="ptrans")
                    nc.tensor.transpose(pt[:], emb_g[:], ident[:])
                    nc.vector.tensor_copy(out=dstT[:, g, :], in_=pt[:])
            for g in range(4):
                nc.vector.tensor_copy(out=xcatT[:, 6, g * 128:(g + 1) * 128],
                                      in_=embT_dec[:, g, :])

        # ---------- precompute Wih@emb gate contributions ----------
        # layout per direction: [128, 6, 512] = (rz 0:4 | i_n 4:6) x (s,b)
        wihf = kload(wihf_d, 1, 768)
        wihb = kload(wihb_d, 1, 768)
        wihe = kload(wihe_d, 1, 768)
        gie_ctx = tc.tile_pool(name="gie", bufs=1)
        gie = gie_ctx.__enter__()
        gi_encF = gie.tile([128, 8, 512], BF16, tag="gi_encF")
        gi_encB = gie.tile([128, 8, 512], BF16, tag="gi_encB")
        nc.vector.memset(gi_encF[:, 4:6, :], 0.0)
        nc.vector.memset(gi_encB[:, 4:6, :], 0.0)
        emb_enc_flat = embT_enc[:].rearrange("p a b -> p (a b)")
        emb_dec_flat = embT_dec[:].rearrange("p a b -> p (a b)")
        with tc.tile_pool(name="pgi", bufs=2, space="PSUM") as pgi:
            def cp(k, out, in_):
                e = (nc.vector, nc.scalar)[k % 2]
                (e.copy if e is nc.scalar else e.tensor_copy)(out=out, in_=in_)
            k = 0
            sub = (0, 1, 2, 3, 6, 7)   # rz -> 0:4, i_n -> 6:8 (4:6 = g_n junk)
            for wih_t, gi in ((wihf, gi_encF), (wihb, gi_encB)):
                for mt in range(6):
                    ps = pgi.tile([128, 512], F32, tag="pgi")
                    nc.tensor.matmul(ps[:], lhsT=wih_t[:, 0, mt * 128:(mt + 1) * 128],
                                     rhs=emb_enc_flat, start=True, stop=True)
                    cp(k, gi[:, sub[mt], :], ps[:])
                    k += 1


        # ---------- encoder: fused fwd+bwd GRU ----------
        # pg layout [128, 6, 16]: rz 0:4 | g_n 4:6 ; cols 0:8 fwd, 8:16 bwd
        # rz preloaded with Wih@emb; Whh n-rows host-prescaled by 0.5 so
        # r*g_n = (th+1)*pg[4:6].
        psE_ctx = tc.tile_pool(name="psE", bufs=2, space="PSUM")
        psE = psE_ctx.__enter__()
        for i in range(S):
            pg = psE.tile([128, 8, 16], F32, tag="epg")
            if i % 8 == 1 and i // 8 < 7:
                # stream one fc-weight k-tile (~5.7us DMA) per 8 encoder
                # steps, on the Pool queue so it runs behind the setup
                # gathers instead of starving them of DMA engines
                kt = i // 8
                nc.gpsimd.dma_start(
                    fcw_sb[:, kt, :],
                    fcwt_d[kt * 128:(kt + 1) * 128, :].rearrange(
                        "(ko p) n -> p (ko n)", p=128))
            for half, whh_t, sp in ((0, whhf, i), (1, whhb, S - 1 - i)):
                cols = slice(half * 8, half * 8 + 8)
                hcol = h_enc[:, :, cols]
                ecol = embT_enc[:, sp // 16, (sp % 16) * 8:(sp % 16) * 8 + 8]
                for mt in range(4):
                    for kt in range(3):
                        nc.tensor.matmul(pg[:, mt, cols],
                                         lhsT=whh_t[:, kt, mt * 128:(mt + 1) * 128],
                                         rhs=hcol[:, kt, :] if kt < 2 else ecol,
                                         start=(kt == 0), stop=(kt == 2),
                                         skip_group_check=True)
                for j in range(2):
                    for kt in range(2):
                        nc.tensor.matmul(pg[:, 4 + j, cols],
                                         lhsT=whh_t[:, kt, (4 + j) * 128:(5 + j) * 128],
                                         rhs=hcol[:, kt, :],
                                         start=(kt == 0), stop=(kt == 1),
                                         skip_group_check=True)
                    nc.tensor.matmul(pg[:, 6 + j, cols],
                                     lhsT=whh_t[:, 2, (4 + j) * 128:(5 + j) * 128],
                                     rhs=ecol, start=True, stop=True,
                                     skip_group_check=True)
            th = stp.tile([128, 4, 16], BF16, tag="e_th")
            nc.scalar.activation(out=th[:], in_=pg[:, 0:4, :], func=AF.Tanh, scale=0.5)
            t_n = stp.tile([128, 2, 16], BF16, tag="e_n")
            nc.vector.scalar_tensor_tensor(out=t_n[:], in0=th[:, 0:2, :], scalar=1.0,
                                           in1=pg[:, 4:6, :], op0=OP.add, op1=OP.mult)
            nc.vector.scalar_tensor_tensor(out=t_n[:], in0=t_n[:], scalar=1.0,
                                           in1=pg[:, 6:8, :], op0=OP.mult, op1=OP.add)
            n_t = stp.tile([128, 2, 16], BF16, tag="e_tanh")
            nc.scalar.activation(out=n_t[:], in_=t_n[:], func=AF.Tanh)
            d_t = stp.tile([128, 2, 16], BF16, tag="e_d")
            nc.vector.tensor_tensor(out=d_t[:], in0=h_enc[:], in1=n_t[:],
                                    op=OP.subtract)
            nc.vector.scalar_tensor_tensor(out=d_t[:], in0=th[:, 2:4, :], scalar=1.0,
                                           in1=d_t[:], op0=OP.add, op1=OP.mult)
            nc.vector.scalar_tensor_tensor(out=h_enc[:], in0=d_t[:], scalar=0.5,
                                           in1=n_t[:], op0=OP.mult, op1=OP.add)
            nc.gpsimd.tensor_copy(out=encT[:, 0:2, i::64], in_=h_enc[:, :, 0:8])
            nc.gpsimd.tensor_copy(out=encT[:, 2:4, (S - 1 - i)::64],
                                  in_=h_enc[:, :, 8:16])

        psE_ctx.__exit__(None, None, None)

        # ---------- h0 + attention precompute ----------
        with tc.tile_pool(name="prep2", bufs=2, space="PSUM") as pp2:
            # hidden0 = tanh(encfc_W @ [hf; hb])
            hcat = stp.tile([128, 4, BL], BF16, tag="hcat")
            nc.vector.tensor_copy(out=hcat[:, 0:2, :], in_=h_enc[:, :, 0:8])
            nc.vector.tensor_copy(out=hcat[:, 2:4, :], in_=h_enc[:, :, 8:16])
            ph0 = pp2.tile([128, 2, BL], F32, tag="ph0")
            for mt in range(2):
                for kt in range(4):
                    nc.tensor.matmul(ph0[:, mt, :],
                                     lhsT=encfc[:, kt, mt * 128:(mt + 1) * 128],
                                     rhs=hcat[:, kt, :], start=(kt == 0), stop=(kt == 3))
            nc.scalar.activation(out=h0_bf[:], in_=ph0[:], func=AF.Tanh)

            for mt in range(2):
                pe = pp2.tile([128, 512], F32, tag="pproj")
                for kt in range(4):
                    nc.tensor.matmul(pe[:], lhsT=we[:, kt, mt * 128:(mt + 1) * 128],
                                     rhs=encT[:, kt, :], start=(kt == 0), stop=(kt == 3))
                nc.vector.tensor_copy(out=enc_proj[:, mt, :], in_=pe[:])
            for et in range(4):
                for bp in range(4):
                    ptp = pp2.tile([128, 128], BF16, tag="ppack")
                    nc.tensor.transpose(ptp[:], encT[:, et, bp * 128:(bp + 1) * 128],
                                        ident[:])
                    nc.vector.tensor_copy(
                        out=enc_pack[:, bp, et * 128:(et + 1) * 128], in_=ptp[:])

        # ---------- decoder + 2D-sharded fc ----------
        psA_ctx = tc.tile_pool(name="psA", bufs=2, space="PSUM")
        psA = psA_ctx.__enter__()
        fcps_ctx = tc.tile_pool(name="fcps", bufs=6, space="PSUM")
        fcps = fcps_ctx.__enter__()
        fcg_ctx = tc.tile_pool(name="fcg", bufs=1)
        fcg_pool = fcg_ctx.__enter__()
        fc_queue = []
        no_pool_steps = set()
        for tf, _, _ in GATHERS:
            no_pool_steps.update(range(tf, tf + 5))
        cur_t = [0]
        fc_eng = [0]
        xg_tiles = {}

        def emit_gather(g):
            _, t0s, nst = GATHERS[g]
            nc.scalar.dma_start(xg_in[g][:],
                                xcatT[:, :, t0s * BL:(t0s + nst) * BL])
            nc.gpsimd.collective_compute(
                "AllGather", OP.bypass,
                replica_groups=[[0, 1, 2, 3], [4, 5, 6, 7]],
                ins=[xg_in[g].ap()], outs=[xg_out[g].ap()])

        def emit_fetch(g):
            # emitted a few steps after the gather so the SEQ wait on the
            # collective is short (a blocked SEQ stalls that engine's queue)
            _, t0s, nst = GATHERS[g]
            xtag = f"xg8{'ab'[g % 2]}" if nst == 8 else f"xg_sb{nst}_{g}"
            # linear DMA (192B runs) then Pool reorder to (t,r,b) for fc lhsT
            xl = fcg_pool.tile([128, 7, NGRP, nst, BL], BF16,
                               tag=xtag + "l", name=f"xl{g}")
            nc.scalar.dma_start(
                xl[:], xg_out[g][:].rearrange("r p k tb -> p k r tb")
                .rearrange("p k r (t b) -> p k r t b", b=BL))
            xg = fcg_pool.tile([128, 7, nst, NGRP, BL], BF16,
                               tag=xtag, name=f"xg{g}")
            for kt in range(KT_X):
                eng = (nc.scalar, nc.vector)[kt % 2]
                (eng.copy if eng is nc.scalar else eng.tensor_copy)(
                    out=xg[:, kt],
                    in_=xl[:, kt].rearrange("p r t b -> p t r b"))
            xg_tiles[g] = xg[:].rearrange("p k t r b -> p k (t r b)")

        def emit_fc_unit(g, mt, ns):
            xg = xg_tiles[g]
            row0 = GATHERS[g][1] * HB + mt * 128
            rows = min(128, M - row0)
            ps = fcps.tile([128, NSUB], F32, tag="fcp")
            for kt in range(KT_X):
                nc.tensor.matmul(
                    ps[:rows, :],
                    lhsT=xg[:, kt, mt * 128:mt * 128 + rows],
                    rhs=fcw_sb[:, kt, ns * NSUB:(ns + 1) * NSUB],
                    start=(kt == 0), stop=(kt == KT_X - 1))
            osb = fco_pool.tile([128, NSUB], BF16, tag="osb")
            eng = (nc.scalar, nc.vector)[fc_eng[0] % 2]
            fc_eng[0] += 1
            (eng.copy if eng is nc.scalar else eng.tensor_copy)(
                out=osb[:rows, :], in_=ps[:rows, :])
            nc.sync.dma_start(
                out_d[row0:row0 + rows, ns * NSUB:(ns + 1) * NSUB],
                osb[:rows, :])

        def pump(k):
            for _ in range(k):
                if fc_queue:
                    emit_fc_unit(*fc_queue.pop(0))

        def dec_step(t):
            h_prev = h0_bf[:] if t == 0 else xcatT[:, 0:2, (t - 1) * 8:t * 8]
            # gates PSUM; rz+i_n preloaded with Wih@emb contribution
            big = psA.tile([128, 16, 8], F32, tag="att")
            pg = big[:, 0:8, :]
            # attention PSUM scratch shares the step tile: pq 8:10 | pw 10:14
            pq = big[:, 8:10, :]
            pw = big[:, 10:14, :]
            psc = big[:, 14, 0:4]
            pz = big[:, 15, 0:4]
            # q^T [DH, 8]
            for mt in range(2):
                for kt in range(2):
                    nc.tensor.matmul(pq[:, mt, :],
                                     lhsT=wh[:, kt, mt * 128:(mt + 1) * 128],
                                     rhs=h_prev[:, kt, :],
                                     start=(kt == 0), stop=(kt == 1))
            # h-dependent gate matmuls (independent of attention)
            for mt in range(4):
                for kt in range(2):
                    nc.tensor.matmul(pg[:, mt, :],
                                     lhsT=whhd[:, kt, mt * 128:(mt + 1) * 128],
                                     rhs=h_prev[:, kt, :],
                                     start=(kt == 0), stop=False,
                                     skip_group_check=True)
            for j in range(2):
                for kt in range(2):
                    nc.tensor.matmul(pg[:, 4 + j, :],
                                     lhsT=whhd[:, kt, (4 + j) * 128:(5 + j) * 128],
                                     rhs=h_prev[:, kt, :],
                                     start=(kt == 0), stop=(kt == 1),
                                     skip_group_check=True)
            q_bf = stp.tile([128, 2, 8], F32, tag="q_bf")
            nc.vector.tensor_copy(out=q_bf[:], in_=pq)
            pump(3)
            # energy = tanh(enc_proj + q): per-(kt,b) adds with q as the
            # per-partition scalar (packed bf16 SBUF operands -> fast DVE)
            energy = stp.tile([128, 2, 512], BF16, tag="energy")
            for h in range(4):
                sl = slice(h * 128, (h + 1) * 128)
                for b in (2 * h, 2 * h + 1):
                    for kt in range(2):
                        nc.vector.tensor_scalar(
                            out=energy[:, kt, b * 64:(b + 1) * 64],
                            in0=enc_proj[:, kt, b * 64:(b + 1) * 64],
                            scalar1=q_bf[:, kt, b:b + 1], scalar2=None,
                            op0=OP.add)
                nc.scalar.activation(out=energy[:, :, sl], in_=energy[:, :, sl],
                                     func=AF.Tanh)
            # scores -> psc [(b,s)-part, j]
            for j in range(4):
                for kt in range(2):
                    nc.tensor.matmul(
                        psc[:, j:j + 1],
                        lhsT=energy[:, kt, j * 128:(j + 1) * 128],
                        rhs=v_sb[:, kt:kt + 1], start=(kt == 0), stop=(kt == 1),
                        skip_group_check=True)
            pump(1)
            exp_f = stp.tile([128, 4], F32, tag="exp_f")
            nc.scalar.activation(out=exp_f[:], in_=psc, func=AF.Exp)
            nc.tensor.matmul(pz, lhsT=blk[:], rhs=exp_f[:], start=True, stop=True)
            rcp = stp.tile([128, 4], F32, tag="rcp")
            nc.vector.reciprocal(out=rcp[:], in_=pz)
            nc.vector.tensor_tensor(out=a_eo[0:64, :, 0], in0=exp_f[0:64, :],
                                    in1=rcp[0:64, :], op=OP.mult)
            nc.vector.tensor_tensor(out=a_eo[64:128, :, 1], in0=exp_f[64:128, :],
                                    in1=rcp[64:128, :], op=OP.mult)
            pump(1)
            # weighted^T [2EH, 8]
            for bp in range(4):
                for et in range(4):
                    nc.tensor.matmul(
                        pw[:, et, 2 * bp:2 * bp + 2],
                        lhsT=enc_pack[:, bp, et * 128:(et + 1) * 128],
                        rhs=a_eo[:, bp, :], start=True, stop=True)
            wdst = xcatT[:, 2:6, t * 8:(t + 1) * 8]
            nc.vector.tensor_copy(out=wdst, in_=pw)
            # w+emb gate matmuls: rhs = xcatT rows 2:7 (w | emb), 5 k-tiles
            xw = xcatT[:, 2:7, t * 8:(t + 1) * 8]
            for mt in range(4):
                for kt in range(5):
                    nc.tensor.matmul(pg[:, mt, :],
                                     lhsT=wihw[:, kt, mt * 128:(mt + 1) * 128],
                                     rhs=xw[:, kt, :],
                                     start=False, stop=(kt == 4),
                                     skip_group_check=True)
            for j in range(2):
                for kt in range(5):
                    nc.tensor.matmul(pg[:, 6 + j, :],
                                     lhsT=wihw[:, kt, (4 + j) * 128:(5 + j) * 128],
                                     rhs=xw[:, kt, :],
                                     start=(kt == 0), stop=(kt == 4),
                                     skip_group_check=True)
            # gates (sigmoid via tanh(x/2); whhd n-rows prescaled by 0.5)
            th = stp.tile([128, 4, 8], BF16, tag="d_th")
            nc.scalar.activation(out=th[:], in_=pg[:, 0:4, :], func=AF.Tanh,
                                 scale=0.5)
            t_n = stp.tile([128, 2, 8], BF16, tag="d_n")
            nc.vector.scalar_tensor_tensor(out=t_n[:], in0=th[:, 0:2, :],
                                           scalar=1.0, in1=pg[:, 4:6, :],
                                           op0=OP.add, op1=OP.mult)
            nc.vector.scalar_tensor_tensor(out=t_n[:], in0=t_n[:], scalar=1.0,
                                           in1=pg[:, 6:8, :], op0=OP.mult,
                                           op1=OP.add)
            n_t = stp.tile([128, 2, 8], BF16, tag="d_tanh")
            nc.scalar.activation(out=n_t[:], in_=t_n[:], func=AF.Tanh)
            d_t = stp.tile([128, 2, 8], BF16, tag="d_d")
            nc.vector.tensor_tensor(out=d_t[:], in0=h_prev, in1=n_t[:],
                                    op=OP.subtract)
            nc.vector.scalar_tensor_tensor(out=d_t[:], in0=th[:, 2:4, :],
                                           scalar=1.0, in1=d_t[:], op0=OP.add,
                                           op1=OP.mult)
            nc.vector.scalar_tensor_tensor(
                out=xcatT[:, 0:2, t * 8:(t + 1) * 8], in0=d_t[:],
                scalar=0.5, in1=n_t[:], op0=OP.mult, op1=OP.add)
            pump(3)

        gather_by_tf = {tf: g for g, (tf, _, _) in enumerate(GATHERS)}
        ready_by_t = {}
        tail_units = []
        for g, (tf, t0s, nst) in enumerate(GATHERS):
            mts = (nst * HB + 127) // 128
            units = [(g, mt, ns) for mt in range(mts) for ns in range(NCH)]
            lag = 8 if nst >= 8 else 7
            if tf + lag <= TD - 1:
                ready_by_t.setdefault(tf + lag, []).extend(units)
            else:
                tail_units.extend(units)

        fetch_by_t = {}
        for g, (tf, _, _) in enumerate(GATHERS):
            if tf + 6 <= TD - 1:
                fetch_by_t[tf + 6] = g
        fetched = set()
        for t in range(TD):
            cur_t[0] = t
            if t in ready_by_t:
                fc_queue.extend(ready_by_t[t])
            dec_step(t)
            if t in fetch_by_t:
                emit_fetch(fetch_by_t[t])
                fetched.add(fetch_by_t[t])
            if t in gather_by_tf:
                emit_gather(gather_by_tf[t])
        cur_t[0] = TD
        no_pool_steps.add(TD)
        for g in range(len(GATHERS)):
            if g not in fetched:
                emit_fetch(g)
        for item in fc_queue + tail_units:
            emit_fc_unit(*item)
        fcg_ctx.__exit__(None, None, None)
        fcps_ctx.__exit__(None, None, None)
        psA_ctx.__exit__(None, None, None)

    nc.compile()
    return nc


def _prep_inputs(inputs):
    """Host-side layout prep shared across cores. Returns (shared, per_core)."""
    f = {k: np.asarray(v) for k, v in inputs.items()}
    bf = lambda a: np.ascontiguousarray(a, dtype=np.float32).astype(bfnp)
    tr = lambda a: bf(np.asarray(a, np.float32).T)

    def half_n(whh):
        w = np.asarray(whh, np.float32).copy()
        w[2 * w.shape[0] // 3:, :] *= 0.5    # prescale n-gate rows
        return tr(w)

    def pk(a):
        a = np.asarray(a, bfnp)
        ko = a.shape[0] // 128
        return a.reshape(ko, 128, a.shape[1]).transpose(1, 0, 2).reshape(128, -1)

    blob = np.concatenate([
        pk(half_n(f["enc_Whh_f"])), pk(half_n(f["enc_Whh_b"])),
        pk(tr(f["enc_fc_W"])),
        pk(tr(f["attn_W"][:, :DH])), pk(tr(f["attn_W"][:, DH:])),
        pk(tr(np.concatenate([f["dec_Wih"][:, E:],
                              f["dec_Wih"][:, :E]], axis=1))),
        pk(half_n(f["dec_Whh"])),
        pk(tr(f["enc_Wih_f"])), pk(tr(f["enc_Wih_b"])),
        pk(tr(f["dec_Wih"][:, :E])),
        bf(f["attn_v"][0].reshape(2, 128).T),
        np.eye(128, dtype=bfnp),
    ], axis=1)
    assert blob.shape == (128, NBLOB), blob.shape

    shared = dict(
        enc_emb=bf(f["enc_emb"]),
        dec_emb=bf(f["dec_emb"]),
        wblob=np.ascontiguousarray(blob),
        blkones=np.kron(np.eye(2, dtype=np.float32), np.ones((64, 64), np.float32)),
    )

    src = np.asarray(f["src"])
    trg = np.asarray(f["trg"])
    fcwt_full = tr(f["fc_W"])                                     # [896, 32000]
    per_core = []
    for c in range(NCORES):
        cols = slice(c * BL, (c + 1) * BL)
        si = src[:, cols].astype(np.int32).reshape(-1)            # s-major, 512
        ti = trg[:TD, cols].astype(np.int32).reshape(-1)          # t-major, 504
        ti = np.concatenate([ti, np.zeros(8, np.int32)])
        tok = np.concatenate([si.reshape(4, 128), ti.reshape(4, 128)]).T  # [128, 8]
        vsh = c % NGRP
        per_core.append(dict(
            tok_idx=np.ascontiguousarray(tok),
            fcw_t=np.ascontiguousarray(fcwt_full[:, vsh * VS:(vsh + 1) * VS])))
    return shared, per_core


def kernel(**inputs):
    if "nc" not in _CACHE:
        _CACHE["nc"] = _build_program()
    nc = _CACHE["nc"]

    shared, per_core = _prep_inputs(inputs)
    in_maps = [{**shared, **pc} for pc in per_core]

    res = run_bass_kernel_spmd(nc, in_maps, core_ids=list(range(NCORES)))
    _CACHE["last_result"] = res

    out = np.zeros((T, B, V), np.float32)
    for c in range(NCORES):
        g, vsh = c // NGRP, c % NGRP
        arr = np.asarray(res.results[c]["out"], dtype=np.float32)
        out[1:, g * HB:(g + 1) * HB, vsh * VS:(vsh + 1) * VS] = \
            arr.reshape(TD, HB, VS)
    return out


# revision 71
# speedup vs baseline: 1.0062x; 1.0062x over previous
"""Trainium2 Bass kernel for GRU seq2seq w/ Bahdanau attention (nn_DSkBart).

Sharding (8 NeuronCores):
  * recurrence (bi-GRU encoder + attention decoder): data-parallel over
    batch, 8 batch cols per core, single fused chain per core in
    transposed-state layout ([feature-on-partitions, batch-on-free]).
  * fc_out: 2D-sharded (vocab x batch-half): core c holds fc_W^T cols
    [(c%4)*8000, ...) resident in SBUF (bf16) and computes logits for
    batch half c//4; decoder states are AllGather'd within each 4-core
    group (halving collective bytes vs world gathers) in staged chunks
    that interleave the big fc matmul with the recurrence on the PE.

Latency/throughput tricks (the kernel is bound by the decoder's serial
dependency chain, with the fc matmul filling PE gaps):
  * sigmoid(x)=(tanh(x/2)+1)/2 keeps one ACT table set; Whh/Wihw n-gate
    rows host-prescaled by 0.5 so r*g_n is one fused DVE op.
  * embedding gate contributions fold into the gate matmuls as extra
    K-tiles ([h; emb] for the encoder, [w; emb] for the decoder) -- no
    separate Wih@emb pass, no PSUM preloads (PSUM preload + accumulate
    proved numerically unsafe on HW).
  * energy adds use per-(kt,b) tensor_scalar with q as the per-partition
    scalar (packed bf16 operands -> fast DVE mode), tanh in 4 quarters
    so score matmuls start early.
  * softmax denominators replicated across partitions via a block-ones
    matmul; all small weights arrive in one packed blob DMA; fc weights
    stream in 7 chunks under the encoder; logits leave in bf16 (host
    converts to f32); collectives are the only Pool-engine work during
    decode (a collective holds its sequencer for the whole transfer,
    and GPSIMD cannot touch PSUM).

Self-contained: hardcodes all shapes; host does layout/dtype prep only.
"""

import numpy as np
import ml_dtypes

import concourse.bass as bass
import concourse.bacc as bacc_mod
import concourse.tile as tile
import concourse.mybir as mybir
from concourse.bass_utils import run_bass_kernel_spmd

# problem dims
V, S, T, B = 32000, 64, 64, 64
E, EH, DH = 128, 256, 256
NCORES = 8
NGRP = 4                  # cores per batch-half group
BL = B // NCORES          # 8 batch cols per core
HB = NGRP * BL            # 32 batch cols per group (batch half)
TD = T - 1                # 63 decoder steps
M = TD * HB               # 2016 output rows per core
KX = DH + 2 * EH + E      # 896 = xcat dim
KT_X = KX // 128          # 7
VS = V // NGRP            # 8000 vocab cols per core
NSUB = 250                # fc psum n-subtile
NCH = VS // NSUB          # 16 n-chunks

F32 = mybir.dt.float32
BF16 = mybir.dt.bfloat16
I32 = mybir.dt.int32
AF = mybir.ActivationFunctionType
OP = mybir.AluOpType
bfnp = ml_dtypes.bfloat16

# (t_issue, t0_start, n_steps); sum of n_steps == 63
GATHERS = ((3, 0, 4), (11, 4, 8), (19, 12, 8), (27, 20, 8), (35, 28, 8),
           (43, 36, 8), (51, 44, 8), (55, 52, 4), (59, 56, 4), (62, 60, 3))

_CACHE = {}


def _build_program():
    nc = bacc_mod.Bacc("TRN2", num_devices=NCORES)

    # ---- DRAM I/O ----
    tok_idx_d = nc.dram_tensor("tok_idx", [128, 8], I32, kind="ExternalInput")
    enc_emb_d = nc.dram_tensor("enc_emb", [V, E], BF16, kind="ExternalInput")
    dec_emb_d = nc.dram_tensor("dec_emb", [V, E], BF16, kind="ExternalInput")
    wihf_d = nc.dram_tensor("wihf_t", [E, 3 * EH], BF16, kind="ExternalInput")
    wihb_d = nc.dram_tensor("wihb_t", [E, 3 * EH], BF16, kind="ExternalInput")
    whhf_d = nc.dram_tensor("whhf_t", [EH, 3 * EH], BF16, kind="ExternalInput")
    whhb_d = nc.dram_tensor("whhb_t", [EH, 3 * EH], BF16, kind="ExternalInput")
    encfc_d = nc.dram_tensor("encfc_t", [2 * EH, DH], BF16, kind="ExternalInput")
    wh_d = nc.dram_tensor("wh_t", [DH, DH], BF16, kind="ExternalInput")
    we_d = nc.dram_tensor("we_t", [2 * EH, DH], BF16, kind="ExternalInput")
    v_d = nc.dram_tensor("v_att", [128, 2], BF16, kind="ExternalInput")
    wihe_d = nc.dram_tensor("wihe_t", [E, 3 * DH], BF16, kind="ExternalInput")
    wihw_d = nc.dram_tensor("wihw_t", [2 * EH, 3 * DH], BF16, kind="ExternalInput")
    whhd_d = nc.dram_tensor("whhd_t", [DH, 3 * DH], BF16, kind="ExternalInput")
    fcwt_d = nc.dram_tensor("fcw_t", [KX, VS], BF16, kind="ExternalInput")
    xg_in = [nc.dram_tensor(f"xg_in{g}", [128, 7, BL * ns], BF16)
             for g, (_, _, ns) in enumerate(GATHERS)]
    xg_out = [nc.dram_tensor(f"xg_out{g}", [NGRP, 128, 7, BL * ns], BF16)
              for g, (_, _, ns) in enumerate(GATHERS)]
    blk_d = nc.dram_tensor("blkones", [128, 128], F32, kind="ExternalInput")
    out_d = nc.dram_tensor("out", [M, VS], BF16, kind="ExternalOutput")

    with tile.TileContext(nc) as tc, \
            tc.tile_pool(name="singles", bufs=1) as sing, \
            tc.tile_pool(name="steps", bufs=3) as stp, \
            tc.tile_pool(name="fcout", bufs=4) as fco_pool:

        def kload(dram, ksub, mdim):
            t = sing.tile([128, ksub, mdim], BF16, tag=dram.name)
            nc.sync.dma_start(t[:], dram[:].rearrange("(ko p) m -> p ko m", p=128))
            return t

        whhf = kload(whhf_d, 2, 768)
        whhb = kload(whhb_d, 2, 768)
        encfc = kload(encfc_d, 4, 256)
        wh = kload(wh_d, 2, 256)
        we = kload(we_d, 4, 256)
        wihw = kload(wihw_d, 4, 768)
        whhd = kload(whhd_d, 2, 768)
        v_sb = sing.tile([128, 2], BF16, tag="v_sb")
        nc.sync.dma_start(v_sb[:], v_d[:])
        ident = sing.tile([128, 128], BF16, tag="ident")
        nc.sync.dma_start(ident[:], ident_d[:])
        blk = sing.tile([128, 128], F32, tag="blk")
        nc.sync.dma_start(blk[:], blk_d[:])
        fcw_sb = sing.tile([128, 7, VS], BF16, tag="fcw_sb")

        # persistent activations
        embT_enc = sing.tile([128, 4, 128], BF16, tag="embT_enc")   # [E,(s,b)]
        embT_dec = sing.tile([128, 4, 128], BF16, tag="embT_dec")   # [E,(t,b)]
        encT = sing.tile([128, 4, 512], BF16, tag="encT")           # [2EH,(b,s)]
        enc_pack = sing.tile([128, 4, 512], BF16, tag="enc_pack")   # [(b%2,s),(b//2,e)]
        enc_proj = sing.tile([128, 2, 512], BF16, tag="enc_proj")   # [DH,(b,s)]
        xcatT = sing.tile([128, 7, 512], BF16, tag="xcatT")         # [896,(t,b)]
        h_enc = sing.tile([128, 2, 16], BF16, tag="h_enc")          # enc f|b state
        h0_bf = sing.tile([128, 2, BL], BF16, tag="h0_bf")
        a_eo = sing.tile([128, 4, 2], BF16, tag="a_eo")

        nc.vector.memset(h_enc[:], 0.0)
        nc.vector.memset(xcatT[:, :, 504:512], 0.0)
        nc.vector.memset(a_eo[:], 0.0)

        # ---------- setup: embedding gathers + transposes ----------
        with tc.tile_pool(name="setup", bufs=4) as setp, \
                tc.tile_pool(name="psetup", bufs=2, space="PSUM") as psetp:
            idx_all = sing.tile([128, 8], I32, tag="idx_all")
            nc.sync.dma_start(idx_all[:], tok_idx_d[:])
            for ti, (table, dstT) in enumerate(((enc_emb_d, embT_enc),
                                                (dec_emb_d, embT_dec))):
                for g in range(4):
                    emb_g = setp.tile([128, 128], BF16, tag="embg")
                    nc.gpsimd.indirect_dma_start(
                        out=emb_g[:], out_offset=None, in_=table[:],
                        in_offset=bass.IndirectOffsetOnAxis(
                            ap=idx_all[:, ti * 4 + g:ti * 4 + g + 1], axis=0))
                    pt = psetp.tile([128, 128], BF16, tag="ptrans")
                    nc.tensor.transpose(pt[:], emb_g[:], ident[:])
                    nc.vector.tensor_copy(out=dstT[:, g, :], in_=pt[:])
            for g in range(4):
                nc.vector.tensor_copy(out=xcatT[:, 6, g * 128:(g + 1) * 128],
                                      in_=embT_dec[:, g, :])

        # ---------- precompute Wih@emb gate contributions ----------
        # layout per direction: [128, 6, 512] = (rz 0:4 | i_n 4:6) x (s,b)
        wihf = kload(wihf_d, 1, 768)
        wihb = kload(wihb_d, 1, 768)
        wihe = kload(wihe_d, 1, 768)
        gie_ctx = tc.tile_pool(name="gie", bufs=1)
        gie = gie_ctx.__enter__()
        gi_encF = gie.tile([128, 8, 512], BF16, tag="gi_encF")
        gi_encB = gie.tile([128, 8, 512], BF16, tag="gi_encB")
        nc.vector.memset(gi_encF[:, 4:6, :], 0.0)
        nc.vector.memset(gi_encB[:, 4:6, :], 0.0)
        emb_enc_flat = embT_enc[:].rearrange("p a b -> p (a b)")
        emb_dec_flat = embT_dec[:].rearrange("p a b -> p (a b)")
        with tc.tile_pool(name="pgi", bufs=2, space="PSUM") as pgi:
            def cp(k, out, in_):
                e = (nc.vector, nc.scalar)[k % 2]
                (e.copy if e is nc.scalar else e.tensor_copy)(out=out, in_=in_)
            k = 0
            sub = (0, 1, 2, 3, 6, 7)   # rz -> 0:4, i_n -> 6:8 (4:6 = g_n junk)
            for wih_t, gi in ((wihf, gi_encF), (wihb, gi_encB)):
                for mt in range(6):
                    ps = pgi.tile([128, 512], F32, tag="pgi")
                    nc.tensor.matmul(ps[:], lhsT=wih_t[:, 0, mt * 128:(mt + 1) * 128],
                                     rhs=emb_enc_flat, start=True, stop=True)
                    cp(k, gi[:, sub[mt], :], ps[:])
                    k += 1


        # ---------- encoder: fused fwd+bwd GRU ----------
        # pg layout [128, 6, 16]: rz 0:4 | g_n 4:6 ; cols 0:8 fwd, 8:16 bwd
        # rz preloaded with Wih@emb; Whh n-rows host-prescaled by 0.5 so
        # r*g_n = (th+1)*pg[4:6].
        psE_ctx = tc.tile_pool(name="psE", bufs=2, space="PSUM")
        psE = psE_ctx.__enter__()
        for i in range(S):
            pg = psE.tile([128, 8, 16], F32, tag="epg")
            if i % 8 == 1 and i // 8 < 7:
                # stream one fc-weight k-tile (~5.7us DMA) per 8 encoder
                # steps, on the Pool queue so it runs behind the setup
                # gathers instead of starving them of DMA engines
                kt = i // 8
                nc.gpsimd.dma_start(
                    fcw_sb[:, kt, :],
                    fcwt_d[kt * 128:(kt + 1) * 128, :].rearrange(
                        "(ko p) n -> p (ko n)", p=128))
            for half, whh_t, sp in ((0, whhf, i), (1, whhb, S - 1 - i)):
                cols = slice(half * 8, half * 8 + 8)
                hcol = h_enc[:, :, cols]
                ecol = embT_enc[:, sp // 16, (sp % 16) * 8:(sp % 16) * 8 + 8]
                for mt in range(4):
                    for kt in range(3):
                        nc.tensor.matmul(pg[:, mt, cols],
                                         lhsT=whh_t[:, kt, mt * 128:(mt + 1) * 128],
                                         rhs=hcol[:, kt, :] if kt < 2 else ecol,
                                         start=(kt == 0), stop=(kt == 2),
                                         skip_group_check=True)
                for j in range(2):
                    for kt in range(2):
                        nc.tensor.matmul(pg[:, 4 + j, cols],
                                         lhsT=whh_t[:, kt, (4 + j) * 128:(5 + j) * 128],
                                         rhs=hcol[:, kt, :],
                                         start=(kt == 0), stop=(kt == 1),
                                         skip_group_check=True)
                    nc.tensor.matmul(pg[:, 6 + j, cols],
                                     lhsT=whh_t[:, 2, (4 + j) * 128:(5 + j) * 128],
                                     rhs=ecol, start=True, stop=True,
                                     skip_group_check=True)
            th = stp.tile([128, 4, 16], BF16, tag="e_th")
            nc.scalar.activation(out=th[:], in_=pg[:, 0:4, :], func=AF.Tanh, scale=0.5)
            t_n = stp.tile([128, 2, 16], BF16, tag="e_n")
            nc.vector.scalar_tensor_tensor(out=t_n[:], in0=th[:, 0:2, :], scalar=1.0,
                                           in1=pg[:, 4:6, :], op0=OP.add, op1=OP.mult)
            nc.vector.scalar_tensor_tensor(out=t_n[:], in0=t_n[:], scalar=1.0,
                                           in1=pg[:, 6:8, :], op0=OP.mult, op1=OP.add)
            n_t = stp.tile([128, 2, 16], BF16, tag="e_tanh")
            nc.scalar.activation(out=n_t[:], in_=t_n[:], func=AF.Tanh)
            d_t = stp.tile([128, 2, 16], BF16, tag="e_d")
            nc.vector.tensor_tensor(out=d_t[:], in0=h_enc[:], in1=n_t[:],
                                    op=OP.subtract)
            nc.vector.scalar_tensor_tensor(out=d_t[:], in0=th[:, 2:4, :], scalar=1.0,
                                           in1=d_t[:], op0=OP.add, op1=OP.mult)
            nc.vector.scalar_tensor_tensor(out=h_enc[:], in0=d_t[:], scalar=0.5,
                                           in1=n_t[:], op0=OP.mult, op1=OP.add)
            nc.gpsimd.tensor_copy(out=encT[:, 0:2, i::64], in_=h_enc[:, :, 0:8])
            nc.gpsimd.tensor_copy(out=encT[:, 2:4, (S - 1 - i)::64],
                                  in_=h_enc[:, :, 8:16])

        psE_ctx.__exit__(None, None, None)

        # ---------- h0 + attention precompute ----------
        with tc.tile_pool(name="prep2", bufs=2, space="PSUM") as pp2:
            # hidden0 = tanh(encfc_W @ [hf; hb])
            hcat = stp.tile([128, 4, BL], BF16, tag="hcat")
            nc.vector.tensor_copy(out=hcat[:, 0:2, :], in_=h_enc[:, :, 0:8])
            nc.vector.tensor_copy(out=hcat[:, 2:4, :], in_=h_enc[:, :, 8:16])
            ph0 = pp2.tile([128, 2, BL], F32, tag="ph0")
            for mt in range(2):
                for kt in range(4):
                    nc.tensor.matmul(ph0[:, mt, :],
                                     lhsT=encfc[:, kt, mt * 128:(mt + 1) * 128],
                                     rhs=hcat[:, kt, :], start=(kt == 0), stop=(kt == 3))
            nc.scalar.activation(out=h0_bf[:], in_=ph0[:], func=AF.Tanh)

            for mt in range(2):
                pe = pp2.tile([128, 512], F32, tag="pproj")
                for kt in range(4):
                    nc.tensor.matmul(pe[:], lhsT=we[:, kt, mt * 128:(mt + 1) * 128],
                                     rhs=encT[:, kt, :], start=(kt == 0), stop=(kt == 3))
                nc.vector.tensor_copy(out=enc_proj[:, mt, :], in_=pe[:])
            for et in range(4):
                for bp in range(4):
                    ptp = pp2.tile([128, 128], BF16, tag="ppack")
                    nc.tensor.transpose(ptp[:], encT[:, et, bp * 128:(bp + 1) * 128],
                                        ident[:])
                    nc.vector.tensor_copy(
                        out=enc_pack[:, bp, et * 128:(et + 1) * 128], in_=ptp[:])

        # ---------- decoder + 2D-sharded fc ----------
        psA_ctx = tc.tile_pool(name="psA", bufs=2, space="PSUM")
        psA = psA_ctx.__enter__()
        fcps_ctx = tc.tile_pool(name="fcps", bufs=6, space="PSUM")
        fcps = fcps_ctx.__enter__()
        fcg_ctx = tc.tile_pool(name="fcg", bufs=1)
        fcg_pool = fcg_ctx.__enter__()
        fc_queue = []
        no_pool_steps = set()
        for tf, _, _ in GATHERS:
            no_pool_steps.update(range(tf, tf + 5))
        cur_t = [0]
        fc_eng = [0]
        xg_tiles = {}

        def emit_gather(g):
            _, t0s, nst = GATHERS[g]
            nc.scalar.dma_start(xg_in[g][:],
                                xcatT[:, :, t0s * BL:(t0s + nst) * BL])
            nc.gpsimd.collective_compute(
                "AllGather", OP.bypass,
                replica_groups=[[0, 1, 2, 3], [4, 5, 6, 7]],
                ins=[xg_in[g].ap()], outs=[xg_out[g].ap()])

        def emit_fetch(g):
            # emitted a few steps after the gather so the SEQ wait on the
            # collective is short (a blocked SEQ stalls that engine's queue)
            _, t0s, nst = GATHERS[g]
            xtag = f"xg8{'ab'[g % 2]}" if nst == 8 else f"xg_sb{nst}_{g}"
            # linear DMA (192B runs) then Pool reorder to (t,r,b) for fc lhsT
            xl = fcg_pool.tile([128, 7, NGRP, nst, BL], BF16,
                               tag=xtag + "l", name=f"xl{g}")
            nc.scalar.dma_start(
                xl[:], xg_out[g][:].rearrange("r p k tb -> p k r tb")
                .rearrange("p k r (t b) -> p k r t b", b=BL))
            xg = fcg_pool.tile([128, 7, nst, NGRP, BL], BF16,
                               tag=xtag, name=f"xg{g}")
            for kt in range(KT_X):
                eng = (nc.scalar, nc.vector)[kt % 2]
                (eng.copy if eng is nc.scalar else eng.tensor_copy)(
                    out=xg[:, kt],
                    in_=xl[:, kt].rearrange("p r t b -> p t r b"))
            xg_tiles[g] = xg[:].rearrange("p k t r b -> p k (t r b)")

        def emit_fc_unit(g, mt, ns):
            xg = xg_tiles[g]
            row0 = GATHERS[g][1] * HB + mt * 128
            rows = min(128, M - row0)
            ps = fcps.tile([128, NSUB], F32, tag="fcp")
            for kt in range(KT_X):
                nc.tensor.matmul(
                    ps[:rows, :],
                    lhsT=xg[:, kt, mt * 128:mt * 128 + rows],
                    rhs=fcw_sb[:, kt, ns * NSUB:(ns + 1) * NSUB],
                    start=(kt == 0), stop=(kt == KT_X - 1))
            osb = fco_pool.tile([128, NSUB], BF16, tag="osb")
            eng = (nc.scalar, nc.vector)[fc_eng[0] % 2]
            fc_eng[0] += 1
            (eng.copy if eng is nc.scalar else eng.tensor_copy)(
                out=osb[:rows, :], in_=ps[:rows, :])
            nc.sync.dma_start(
                out_d[row0:row0 + rows, ns * NSUB:(ns + 1) * NSUB],
                osb[:rows, :])

        def pump(k):
            for _ in range(k):
                if fc_queue:
                    emit_fc_unit(*fc_queue.pop(0))

        def dec_step(t):
            h_prev = h0_bf[:] if t == 0 else xcatT[:, 0:2, (t - 1) * 8:t * 8]
            # gates PSUM; rz+i_n preloaded with Wih@emb contribution
            big = psA.tile([128, 16, 8], F32, tag="att")
            pg = big[:, 0:8, :]
            # attention PSUM scratch shares the step tile: pq 8:10 | pw 10:14
            pq = big[:, 8:10, :]
            pw = big[:, 10:14, :]
            psc = big[:, 14, 0:4]
            pz = big[:, 15, 0:4]
            # q^T [DH, 8]
            for mt in range(2):
                for kt in range(2):
                    nc.tensor.matmul(pq[:, mt, :],
                                     lhsT=wh[:, kt, mt * 128:(mt + 1) * 128],
                                     rhs=h_prev[:, kt, :],
                                     start=(kt == 0), stop=(kt == 1))
            # h-dependent gate matmuls (independent of attention)
            for mt in range(4):
                for kt in range(2):
                    nc.tensor.matmul(pg[:, mt, :],
                                     lhsT=whhd[:, kt, mt * 128:(mt + 1) * 128],
                                     rhs=h_prev[:, kt, :],
                                     start=(kt == 0), stop=False,
                                     skip_group_check=True)
            for j in range(2):
                for kt in range(2):
                    nc.tensor.matmul(pg[:, 4 + j, :],
                                     lhsT=whhd[:, kt, (4 + j) * 128:(5 + j) * 128],
                                     rhs=h_prev[:, kt, :],
                                     start=(kt == 0), stop=(kt == 1),
                                     skip_group_check=True)
            q_bf = stp.tile([128, 2, 8], F32, tag="q_bf")
            nc.vector.tensor_copy(out=q_bf[:], in_=pq)
            pump(3)
            # energy = tanh(enc_proj + q): per-(kt,b) adds with q as the
            # per-partition scalar (packed bf16 SBUF operands -> fast DVE)
            energy = stp.tile([128, 2, 512], BF16, tag="energy")
            for h in range(4):
                sl = slice(h * 128, (h + 1) * 128)
                for b in (2 * h, 2 * h + 1):
                    for kt in range(2):
                        nc.vector.tensor_scalar(
                            out=energy[:, kt, b * 64:(b + 1) * 64],
                            in0=enc_proj[:, kt, b * 64:(b + 1) * 64],
                            scalar1=q_bf[:, kt, b:b + 1], scalar2=None,
                            op0=OP.add)
                nc.scalar.activation(out=energy[:, :, sl], in_=energy[:, :, sl],
                                     func=AF.Tanh)
            # scores -> psc [(b,s)-part, j]
            for j in range(4):
                for kt in range(2):
                    nc.tensor.matmul(
                        psc[:, j:j + 1],
                        lhsT=energy[:, kt, j * 128:(j + 1) * 128],
                        rhs=v_sb[:, kt:kt + 1], start=(kt == 0), stop=(kt == 1),
                        skip_group_check=True)
            pump(1)
            exp_f = stp.tile([128, 4], F32, tag="exp_f")
            nc.scalar.activation(out=exp_f[:], in_=psc, func=AF.Exp)
            nc.tensor.matmul(pz, lhsT=blk[:], rhs=exp_f[:], start=True, stop=True)
            rcp = stp.tile([128, 4], F32, tag="rcp")
            nc.vector.reciprocal(out=rcp[:], in_=pz)
            nc.vector.tensor_tensor(out=a_eo[0:64, :, 0], in0=exp_f[0:64, :],
                                    in1=rcp[0:64, :], op=OP.mult)
            nc.vector.tensor_tensor(out=a_eo[64:128, :, 1], in0=exp_f[64:128, :],
                                    in1=rcp[64:128, :], op=OP.mult)
            pump(1)
            # weighted^T [2EH, 8]
            for bp in range(4):
                for et in range(4):
                    nc.tensor.matmul(
                        pw[:, et, 2 * bp:2 * bp + 2],
                        lhsT=enc_pack[:, bp, et * 128:(et + 1) * 128],
                        rhs=a_eo[:, bp, :], start=True, stop=True)
            wdst = xcatT[:, 2:6, t * 8:(t + 1) * 8]
            nc.vector.tensor_copy(out=wdst, in_=pw)
            # w+emb gate matmuls: rhs = xcatT rows 2:7 (w | emb), 5 k-tiles
            xw = xcatT[:, 2:7, t * 8:(t + 1) * 8]
            for mt in range(4):
                for kt in range(5):
                    nc.tensor.matmul(pg[:, mt, :],
                                     lhsT=wihw[:, kt, mt * 128:(mt + 1) * 128],
                                     rhs=xw[:, kt, :],
                                     start=False, stop=(kt == 4),
                                     skip_group_check=True)
            for j in range(2):
                for kt in range(5):
                    nc.tensor.matmul(pg[:, 6 + j, :],
                                     lhsT=wihw[:, kt, (4 + j) * 128:(5 + j) * 128],
                                     rhs=xw[:, kt, :],
                                     start=(kt == 0), stop=(kt == 4),
                                     skip_group_check=True)
            # gates (sigmoid via tanh(x/2); whhd n-rows prescaled by 0.5)
            th = stp.tile([128, 4, 8], BF16, tag="d_th")
            nc.scalar.activation(out=th[:], in_=pg[:, 0:4, :], func=AF.Tanh,
                                 scale=0.5)
            t_n = stp.tile([128, 2, 8], BF16, tag="d_n")
            nc.vector.scalar_tensor_tensor(out=t_n[:], in0=th[:, 0:2, :],
                                           scalar=1.0, in1=pg[:, 4:6, :],
                                           op0=OP.add, op1=OP.mult)
            nc.vector.scalar_tensor_tensor(out=t_n[:], in0=t_n[:], scalar=1.0,
                                           in1=pg[:, 6:8, :], op0=OP.mult,
                                           op1=OP.add)
            n_t = stp.tile([128, 2, 8], BF16, tag="d_tanh")
            nc.scalar.activation(out=n_t[:], in_=t_n[:], func=AF.Tanh)
            d_t = stp.tile([128, 2, 8], BF16, tag="d_d")
            nc.vector.tensor_tensor(out=d_t[:], in0=h_prev, in1=n_t[:],
                                    op=OP.subtract)
            nc.vector.scalar_tensor_tensor(out=d_t[:], in0=th[:, 2:4, :],
                                           scalar=1.0, in1=d_t[:], op0=OP.add,
                                           op1=OP.mult)
            nc.vector.scalar_tensor_tensor(
                out=xcatT[:, 0:2, t * 8:(t + 1) * 8], in0=d_t[:],
                scalar=0.5, in1=n_t[:], op0=OP.mult, op1=OP.add)
            pump(3)

        gather_by_tf = {tf: g for g, (tf, _, _) in enumerate(GATHERS)}
        ready_by_t = {}
        tail_units = []
        for g, (tf, t0s, nst) in enumerate(GATHERS):
            mts = (nst * HB + 127) // 128
            units = [(g, mt, ns) for mt in range(mts) for ns in range(NCH)]
            lag = 7 if nst >= 8 else 7
            if tf + lag <= TD - 1:
                ready_by_t.setdefault(tf + lag, []).extend(units)
            else:
                tail_units.extend(units)

        fetch_by_t = {}
        for g, (tf, _, _) in enumerate(GATHERS):
            if tf + 6 <= TD - 1:
                fetch_by_t[tf + 6] = g
        fetched = set()
        for t in range(TD):
            cur_t[0] = t
            if t in ready_by_t:
                fc_queue.extend(ready_by_t[t])
            dec_step(t)
            if t in fetch_by_t:
                emit_fetch(fetch_by_t[t])
                fetched.add(fetch_by_t[t])
            if t in gather_by_tf:
                emit_gather(gather_by_tf[t])
        cur_t[0] = TD
        no_pool_steps.add(TD)
        for g in range(len(GATHERS)):
            if g not in fetched:
                emit_fetch(g)
        for item in fc_queue + tail_units:
            emit_fc_unit(*item)
        fcg_ctx.__exit__(None, None, None)
        fcps_ctx.__exit__(None, None, None)
        psA_ctx.__exit__(None, None, None)

    nc.compile()
    return nc


def _prep_inputs(inputs):
    """Host-side layout prep shared across cores. Returns (shared, per_core)."""
    f = {k: np.asarray(v) for k, v in inputs.items()}
    bf = lambda a: np.ascontiguousarray(a, dtype=np.float32).astype(bfnp)
    tr = lambda a: bf(np.asarray(a, np.float32).T)

    def half_n(whh):
        w = np.asarray(whh, np.float32).copy()
        w[2 * w.shape[0] // 3:, :] *= 0.5    # prescale n-gate rows
        return tr(w)

    def pk(a):
        a = np.asarray(a, bfnp)
        ko = a.shape[0] // 128
        return a.reshape(ko, 128, a.shape[1]).transpose(1, 0, 2).reshape(128, -1)

    blob = np.concatenate([
        pk(half_n(f["enc_Whh_f"])), pk(half_n(f["enc_Whh_b"])),
        pk(tr(f["enc_fc_W"])),
        pk(tr(f["attn_W"][:, :DH])), pk(tr(f["attn_W"][:, DH:])),
        pk(tr(np.concatenate([f["dec_Wih"][:, E:],
                              f["dec_Wih"][:, :E]], axis=1))),
        pk(half_n(f["dec_Whh"])),
        pk(tr(f["enc_Wih_f"])), pk(tr(f["enc_Wih_b"])),
        pk(tr(f["dec_Wih"][:, :E])),
        bf(f["attn_v"][0].reshape(2, 128).T),
        np.eye(128, dtype=bfnp),
    ], axis=1)
    assert blob.shape == (128, NBLOB), blob.shape

    shared = dict(
        enc_emb=bf(f["enc_emb"]),
        dec_emb=bf(f["dec_emb"]),
        wblob=np.ascontiguousarray(blob),
        blkones=np.kron(np.eye(2, dtype=np.float32), np.ones((64, 64), np.float32)),
    )

    src = np.asarray(f["src"])
    trg = np.asarray(f["trg"])
    fcwt_full = tr(f["fc_W"])                                     # [896, 32000]
    per_core = []
    for c in range(NCORES):
        cols = slice(c * BL, (c + 1) * BL)
        si = src[:, cols].astype(np.int32).reshape(-1)            # s-major, 512
        ti = trg[:TD, cols].astype(np.int32).reshape(-1)          # t-major, 504
        ti = np.concatenate([ti, np.zeros(8, np.int32)])
        tok = np.concatenate([si.reshape(4, 128), ti.reshape(4, 128)]).T  # [128, 8]
        vsh = c % NGRP
        per_core.append(dict(
            tok_idx=np.ascontiguousarray(tok),
            fcw_t=np.ascontiguousarray(fcwt_full[:, vsh * VS:(vsh + 1) * VS])))
    return shared, per_core


def kernel(**inputs):
    if "nc" not in _CACHE:
        _CACHE["nc"] = _build_program()
    nc = _CACHE["nc"]

    shared, per_core = _prep_inputs(inputs)
    in_maps = [{**shared, **pc} for pc in per_core]

    res = run_bass_kernel_spmd(nc, in_maps, core_ids=list(range(NCORES)))
    _CACHE["last_result"] = res

    out = np.zeros((T, B, V), np.float32)
    for c in range(NCORES):
        g, vsh = c // NGRP, c % NGRP
        arr = np.asarray(res.results[c]["out"], dtype=np.float32)
        out[1:, g * HB:(g + 1) * HB, vsh * VS:(vsh + 1) * VS] = \
            arr.reshape(TD, HB, VS)
    return out


# revision 73
# speedup vs baseline: 1.0066x; 1.0003x over previous
"""Trainium2 Bass kernel for GRU seq2seq w/ Bahdanau attention (nn_DSkBart).

Sharding (8 NeuronCores):
  * recurrence (bi-GRU encoder + attention decoder): data-parallel over
    batch, 8 batch cols per core, single fused chain per core in
    transposed-state layout ([feature-on-partitions, batch-on-free]).
  * fc_out: 2D-sharded (vocab x batch-half): core c holds fc_W^T cols
    [(c%4)*8000, ...) resident in SBUF (bf16) and computes logits for
    batch half c//4; decoder states are AllGather'd within each 4-core
    group (halving collective bytes vs world gathers) in staged chunks
    that interleave the big fc matmul with the recurrence on the PE.

Latency/throughput tricks (the kernel is bound by the decoder's serial
dependency chain, with the fc matmul filling PE gaps):
  * sigmoid(x)=(tanh(x/2)+1)/2 keeps one ACT table set; Whh/Wihw n-gate
    rows host-prescaled by 0.5 so r*g_n is one fused DVE op.
  * embedding gate contributions fold into the gate matmuls as extra
    K-tiles ([h; emb] for the encoder, [w; emb] for the decoder) -- no
    separate Wih@emb pass, no PSUM preloads (PSUM preload + accumulate
    proved numerically unsafe on HW).
  * energy adds use per-(kt,b) tensor_scalar with q as the per-partition
    scalar (packed bf16 operands -> fast DVE mode), tanh in 4 quarters
    so score matmuls start early.
  * softmax denominators replicated across partitions via a block-ones
    matmul; all small weights arrive in one packed blob DMA; fc weights
    stream in 7 chunks under the encoder; logits leave in bf16 (host
    converts to f32); collectives are the only Pool-engine work during
    decode (a collective holds its sequencer for the whole transfer,
    and GPSIMD cannot touch PSUM).

Self-contained: hardcodes all shapes; host does layout/dtype prep only.
"""

import numpy as np
import ml_dtypes

import concourse.bass as bass
import concourse.bacc as bacc_mod
import concourse.tile as tile
import concourse.mybir as mybir
from concourse.bass_utils import run_bass_kernel_spmd

# problem dims
V, S, T, B = 32000, 64, 64, 64
E, EH, DH = 128, 256, 256
NCORES = 8
NGRP = 4                  # cores per batch-half group
BL = B // NCORES          # 8 batch cols per core
HB = NGRP * BL            # 32 batch cols per group (batch half)
TD = T - 1                # 63 decoder steps
M = TD * HB               # 2016 output rows per core
KX = DH + 2 * EH + E      # 896 = xcat dim
KT_X = KX // 128          # 7
VS = V // NGRP            # 8000 vocab cols per core
NSUB = 250                # fc psum n-subtile
NCH = VS // NSUB          # 16 n-chunks

F32 = mybir.dt.float32
BF16 = mybir.dt.bfloat16
I32 = mybir.dt.int32
AF = mybir.ActivationFunctionType
OP = mybir.AluOpType
bfnp = ml_dtypes.bfloat16

# (t_issue, t0_start, n_steps); sum of n_steps == 63
GATHERS = ((3, 0, 4), (11, 4, 8), (19, 12, 8), (27, 20, 8), (35, 28, 8),
           (43, 36, 8), (51, 44, 8), (55, 52, 4), (59, 56, 4), (62, 60, 3))

_CACHE = {}


def _build_program():
    nc = bacc_mod.Bacc("TRN2", num_devices=NCORES)

    # ---- DRAM I/O ----
    tok_idx_d = nc.dram_tensor("tok_idx", [128, 8], I32, kind="ExternalInput")
    enc_emb_d = nc.dram_tensor("enc_emb", [V, E], BF16, kind="ExternalInput")
    dec_emb_d = nc.dram_tensor("dec_emb", [V, E], BF16, kind="ExternalInput")
    wihf_d = nc.dram_tensor("wihf_t", [E, 3 * EH], BF16, kind="ExternalInput")
    wihb_d = nc.dram_tensor("wihb_t", [E, 3 * EH], BF16, kind="ExternalInput")
    whhf_d = nc.dram_tensor("whhf_t", [EH, 3 * EH], BF16, kind="ExternalInput")
    whhb_d = nc.dram_tensor("whhb_t", [EH, 3 * EH], BF16, kind="ExternalInput")
    encfc_d = nc.dram_tensor("encfc_t", [2 * EH, DH], BF16, kind="ExternalInput")
    wh_d = nc.dram_tensor("wh_t", [DH, DH], BF16, kind="ExternalInput")
    we_d = nc.dram_tensor("we_t", [2 * EH, DH], BF16, kind="ExternalInput")
    v_d = nc.dram_tensor("v_att", [128, 2], BF16, kind="ExternalInput")
    wihe_d = nc.dram_tensor("wihe_t", [E, 3 * DH], BF16, kind="ExternalInput")
    wihw_d = nc.dram_tensor("wihw_t", [2 * EH, 3 * DH], BF16, kind="ExternalInput")
    whhd_d = nc.dram_tensor("whhd_t", [DH, 3 * DH], BF16, kind="ExternalInput")
    fcwt_d = nc.dram_tensor("fcw_t", [KX, VS], BF16, kind="ExternalInput")
    xg_in = [nc.dram_tensor(f"xg_in{g}", [128, 7, BL * ns], BF16)
             for g, (_, _, ns) in enumerate(GATHERS)]
    xg_out = [nc.dram_tensor(f"xg_out{g}", [NGRP, 128, 7, BL * ns], BF16)
              for g, (_, _, ns) in enumerate(GATHERS)]
    blk_d = nc.dram_tensor("blkones", [128, 128], F32, kind="ExternalInput")
    out_d = nc.dram_tensor("out", [M, VS], BF16, kind="ExternalOutput")

    with tile.TileContext(nc) as tc, \
            tc.tile_pool(name="singles", bufs=1) as sing, \
            tc.tile_pool(name="steps", bufs=3) as stp, \
            tc.tile_pool(name="fcout", bufs=4) as fco_pool:

        def kload(dram, ksub, mdim):
            t = sing.tile([128, ksub, mdim], BF16, tag=dram.name)
            nc.sync.dma_start(t[:], dram[:].rearrange("(ko p) m -> p ko m", p=128))
            return t

        whhf = kload(whhf_d, 2, 768)
        whhb = kload(whhb_d, 2, 768)
        encfc = kload(encfc_d, 4, 256)
        wh = kload(wh_d, 2, 256)
        we = kload(we_d, 4, 256)
        wihw = kload(wihw_d, 4, 768)
        whhd = kload(whhd_d, 2, 768)
        v_sb = sing.tile([128, 2], BF16, tag="v_sb")
        nc.sync.dma_start(v_sb[:], v_d[:])
        ident = sing.tile([128, 128], BF16, tag="ident")
        nc.sync.dma_start(ident[:], ident_d[:])
        blk = sing.tile([128, 128], F32, tag="blk")
        nc.sync.dma_start(blk[:], blk_d[:])
        fcw_sb = sing.tile([128, 7, VS], BF16, tag="fcw_sb")

        # persistent activations
        embT_enc = sing.tile([128, 4, 128], BF16, tag="embT_enc")   # [E,(s,b)]
        embT_dec = sing.tile([128, 4, 128], BF16, tag="embT_dec")   # [E,(t,b)]
        encT = sing.tile([128, 4, 512], BF16, tag="encT")           # [2EH,(b,s)]
        enc_pack = sing.tile([128, 4, 512], BF16, tag="enc_pack")   # [(b%2,s),(b//2,e)]
        enc_proj = sing.tile([128, 2, 512], BF16, tag="enc_proj")   # [DH,(b,s)]
        xcatT = sing.tile([128, 7, 512], BF16, tag="xcatT")         # [896,(t,b)]
        h_enc = sing.tile([128, 2, 16], BF16, tag="h_enc")          # enc f|b state
        h0_bf = sing.tile([128, 2, BL], BF16, tag="h0_bf")
        a_eo = sing.tile([128, 4, 2], BF16, tag="a_eo")

        nc.vector.memset(h_enc[:], 0.0)
        nc.vector.memset(xcatT[:, :, 504:512], 0.0)
        nc.vector.memset(a_eo[:], 0.0)

        # ---------- setup: embedding gathers + transposes ----------
        with tc.tile_pool(name="setup", bufs=4) as setp, \
                tc.tile_pool(name="psetup", bufs=2, space="PSUM") as psetp:
            idx_all = sing.tile([128, 8], I32, tag="idx_all")
            nc.sync.dma_start(idx_all[:], tok_idx_d[:])
            for ti, (table, dstT) in enumerate(((enc_emb_d, embT_enc),
                                                (dec_emb_d, embT_dec))):
                for g in range(4):
                    emb_g = setp.tile([128, 128], BF16, tag="embg")
                    nc.gpsimd.indirect_dma_start(
                        out=emb_g[:], out_offset=None, in_=table[:],
                        in_offset=bass.IndirectOffsetOnAxis(
                            ap=idx_all[:, ti * 4 + g:ti * 4 + g + 1], axis=0))
                    pt = psetp.tile([128, 128], BF16, tag="ptrans")
                    nc.tensor.transpose(pt[:], emb_g[:], ident[:])
                    nc.vector.tensor_copy(out=dstT[:, g, :], in_=pt[:])
            for g in range(4):
                nc.vector.tensor_copy(out=xcatT[:, 6, g * 128:(g + 1) * 128],
                                      in_=embT_dec[:, g, :])

        # ---------- precompute Wih@emb gate contributions ----------
        # layout per direction: [128, 6, 512] = (rz 0:4 | i_n 4:6) x (s,b)
        wihf = kload(wihf_d, 1, 768)
        wihb = kload(wihb_d, 1, 768)
        wihe = kload(wihe_d, 1, 768)
        gie_ctx = tc.tile_pool(name="gie", bufs=1)
        gie = gie_ctx.__enter__()
        gi_encF = gie.tile([128, 8, 512], BF16, tag="gi_encF")
        gi_encB = gie.tile([128, 8, 512], BF16, tag="gi_encB")
        nc.vector.memset(gi_encF[:, 4:6, :], 0.0)
        nc.vector.memset(gi_encB[:, 4:6, :], 0.0)
        emb_enc_flat = embT_enc[:].rearrange("p a b -> p (a b)")
        emb_dec_flat = embT_dec[:].rearrange("p a b -> p (a b)")
        with tc.tile_pool(name="pgi", bufs=2, space="PSUM") as pgi:
            def cp(k, out, in_):
                e = (nc.vector, nc.scalar)[k % 2]
                (e.copy if e is nc.scalar else e.tensor_copy)(out=out, in_=in_)
            k = 0
            sub = (0, 1, 2, 3, 6, 7)   # rz -> 0:4, i_n -> 6:8 (4:6 = g_n junk)
            for wih_t, gi in ((wihf, gi_encF), (wihb, gi_encB)):
                for mt in range(6):
                    ps = pgi.tile([128, 512], F32, tag="pgi")
                    nc.tensor.matmul(ps[:], lhsT=wih_t[:, 0, mt * 128:(mt + 1) * 128],
                                     rhs=emb_enc_flat, start=True, stop=True)
                    cp(k, gi[:, sub[mt], :], ps[:])
                    k += 1


        # ---------- encoder: fused fwd+bwd GRU ----------
        # pg layout [128, 6, 16]: rz 0:4 | g_n 4:6 ; cols 0:8 fwd, 8:16 bwd
        # rz preloaded with Wih@emb; Whh n-rows host-prescaled by 0.5 so
        # r*g_n = (th+1)*pg[4:6].
        psE_ctx = tc.tile_pool(name="psE", bufs=2, space="PSUM")
        psE = psE_ctx.__enter__()
        for i in range(S):
            pg = psE.tile([128, 8, 16], F32, tag="epg")
            if i % 8 == 1 and i // 8 < 7:
                # stream one fc-weight k-tile (~5.7us DMA) per 8 encoder
                # steps, on the Pool queue so it runs behind the setup
                # gathers instead of starving them of DMA engines
                kt = i // 8
                nc.gpsimd.dma_start(
                    fcw_sb[:, kt, :],
                    fcwt_d[kt * 128:(kt + 1) * 128, :].rearrange(
                        "(ko p) n -> p (ko n)", p=128))
            for half, whh_t, sp in ((0, whhf, i), (1, whhb, S - 1 - i)):
                cols = slice(half * 8, half * 8 + 8)
                hcol = h_enc[:, :, cols]
                ecol = embT_enc[:, sp // 16, (sp % 16) * 8:(sp % 16) * 8 + 8]
                for mt in range(4):
                    for kt in range(3):
                        nc.tensor.matmul(pg[:, mt, cols],
                                         lhsT=whh_t[:, kt, mt * 128:(mt + 1) * 128],
                                         rhs=hcol[:, kt, :] if kt < 2 else ecol,
                                         start=(kt == 0), stop=(kt == 2),
                                         skip_group_check=True)
                for j in range(2):
                    for kt in range(2):
                        nc.tensor.matmul(pg[:, 4 + j, cols],
                                         lhsT=whh_t[:, kt, (4 + j) * 128:(5 + j) * 128],
                                         rhs=hcol[:, kt, :],
                                         start=(kt == 0), stop=(kt == 1),
                                         skip_group_check=True)
                    nc.tensor.matmul(pg[:, 6 + j, cols],
                                     lhsT=whh_t[:, 2, (4 + j) * 128:(5 + j) * 128],
                                     rhs=ecol, start=True, stop=True,
                                     skip_group_check=True)
            th = stp.tile([128, 4, 16], BF16, tag="e_th")
            nc.scalar.activation(out=th[:], in_=pg[:, 0:4, :], func=AF.Tanh, scale=0.5)
            t_n = stp.tile([128, 2, 16], BF16, tag="e_n")
            nc.vector.scalar_tensor_tensor(out=t_n[:], in0=th[:, 0:2, :], scalar=1.0,
                                           in1=pg[:, 4:6, :], op0=OP.add, op1=OP.mult)
            nc.vector.scalar_tensor_tensor(out=t_n[:], in0=t_n[:], scalar=1.0,
                                           in1=pg[:, 6:8, :], op0=OP.mult, op1=OP.add)
            n_t = stp.tile([128, 2, 16], BF16, tag="e_tanh")
            nc.scalar.activation(out=n_t[:], in_=t_n[:], func=AF.Tanh)
            d_t = stp.tile([128, 2, 16], BF16, tag="e_d")
            nc.vector.tensor_tensor(out=d_t[:], in0=h_enc[:], in1=n_t[:],
                                    op=OP.subtract)
            nc.vector.scalar_tensor_tensor(out=d_t[:], in0=th[:, 2:4, :], scalar=1.0,
                                           in1=d_t[:], op0=OP.add, op1=OP.mult)
            nc.vector.scalar_tensor_tensor(out=h_enc[:], in0=d_t[:], scalar=0.5,
                                           in1=n_t[:], op0=OP.mult, op1=OP.add)
            nc.gpsimd.tensor_copy(out=encT[:, 0:2, i::64], in_=h_enc[:, :, 0:8])
            nc.gpsimd.tensor_copy(out=encT[:, 2:4, (S - 1 - i)::64],
                                  in_=h_enc[:, :, 8:16])

        psE_ctx.__exit__(None, None, None)

        # ---------- h0 + attention precompute ----------
        with tc.tile_pool(name="prep2", bufs=2, space="PSUM") as pp2:
            # hidden0 = tanh(encfc_W @ [hf; hb])
            hcat = stp.tile([128, 4, BL], BF16, tag="hcat")
            nc.vector.tensor_copy(out=hcat[:, 0:2, :], in_=h_enc[:, :, 0:8])
            nc.vector.tensor_copy(out=hcat[:, 2:4, :], in_=h_enc[:, :, 8:16])
            ph0 = pp2.tile([128, 2, BL], F32, tag="ph0")
            for mt in range(2):
                for kt in range(4):
                    nc.tensor.matmul(ph0[:, mt, :],
                                     lhsT=encfc[:, kt, mt * 128:(mt + 1) * 128],
                                     rhs=hcat[:, kt, :], start=(kt == 0), stop=(kt == 3))
            nc.scalar.activation(out=h0_bf[:], in_=ph0[:], func=AF.Tanh)

            for mt in range(2):
                pe = pp2.tile([128, 512], F32, tag="pproj")
                for kt in range(4):
                    nc.tensor.matmul(pe[:], lhsT=we[:, kt, mt * 128:(mt + 1) * 128],
                                     rhs=encT[:, kt, :], start=(kt == 0), stop=(kt == 3))
                nc.vector.tensor_copy(out=enc_proj[:, mt, :], in_=pe[:])
            for et in range(4):
                for bp in range(4):
                    ptp = pp2.tile([128, 128], BF16, tag="ppack")
                    nc.tensor.transpose(ptp[:], encT[:, et, bp * 128:(bp + 1) * 128],
                                        ident[:])
                    nc.vector.tensor_copy(
                        out=enc_pack[:, bp, et * 128:(et + 1) * 128], in_=ptp[:])

        # ---------- decoder + 2D-sharded fc ----------
        psA_ctx = tc.tile_pool(name="psA", bufs=2, space="PSUM")
        psA = psA_ctx.__enter__()
        fcps_ctx = tc.tile_pool(name="fcps", bufs=6, space="PSUM")
        fcps = fcps_ctx.__enter__()
        fcg_ctx = tc.tile_pool(name="fcg", bufs=1)
        fcg_pool = fcg_ctx.__enter__()
        fc_queue = []
        no_pool_steps = set()
        for tf, _, _ in GATHERS:
            no_pool_steps.update(range(tf, tf + 5))
        cur_t = [0]
        fc_eng = [0]
        xg_tiles = {}

        def emit_gather(g):
            _, t0s, nst = GATHERS[g]
            nc.scalar.dma_start(xg_in[g][:],
                                xcatT[:, :, t0s * BL:(t0s + nst) * BL])
            nc.gpsimd.collective_compute(
                "AllGather", OP.bypass,
                replica_groups=[[0, 1, 2, 3], [4, 5, 6, 7]],
                ins=[xg_in[g].ap()], outs=[xg_out[g].ap()])

        def emit_fetch(g):
            # emitted a few steps after the gather so the SEQ wait on the
            # collective is short (a blocked SEQ stalls that engine's queue)
            _, t0s, nst = GATHERS[g]
            xtag = f"xg8{'ab'[g % 2]}" if nst == 8 else f"xg_sb{nst}_{g}"
            # linear DMA (192B runs) then Pool reorder to (t,r,b) for fc lhsT
            xl = fcg_pool.tile([128, 7, NGRP, nst, BL], BF16,
                               tag=xtag + "l", name=f"xl{g}")
            nc.scalar.dma_start(
                xl[:], xg_out[g][:].rearrange("r p k tb -> p k r tb")
                .rearrange("p k r (t b) -> p k r t b", b=BL))
            xg = fcg_pool.tile([128, 7, nst, NGRP, BL], BF16,
                               tag=xtag, name=f"xg{g}")
            for kt in range(KT_X):
                eng = (nc.scalar, nc.vector)[kt % 2]
                (eng.copy if eng is nc.scalar else eng.tensor_copy)(
                    out=xg[:, kt],
                    in_=xl[:, kt].rearrange("p r t b -> p t r b"))
            xg_tiles[g] = xg[:].rearrange("p k t r b -> p k (t r b)")

        def emit_fc_unit(g, mt, ns):
            xg = xg_tiles[g]
            row0 = GATHERS[g][1] * HB + mt * 128
            rows = min(128, M - row0)
            ps = fcps.tile([128, NSUB], F32, tag="fcp")
            for kt in range(KT_X):
                nc.tensor.matmul(
                    ps[:rows, :],
                    lhsT=xg[:, kt, mt * 128:mt * 128 + rows],
                    rhs=fcw_sb[:, kt, ns * NSUB:(ns + 1) * NSUB],
                    start=(kt == 0), stop=(kt == KT_X - 1))
            osb = fco_pool.tile([128, NSUB], BF16, tag="osb")
            eng = (nc.scalar, nc.vector)[fc_eng[0] % 2]
            fc_eng[0] += 1
            (eng.copy if eng is nc.scalar else eng.tensor_copy)(
                out=osb[:rows, :], in_=ps[:rows, :])
            nc.sync.dma_start(
                out_d[row0:row0 + rows, ns * NSUB:(ns + 1) * NSUB],
                osb[:rows, :])

        def pump(k):
            for _ in range(k):
                if fc_queue:
                    emit_fc_unit(*fc_queue.pop(0))

        def dec_step(t):
            h_prev = h0_bf[:] if t == 0 else xcatT[:, 0:2, (t - 1) * 8:t * 8]
            # gates PSUM; rz+i_n preloaded with Wih@emb contribution
            big = psA.tile([128, 16, 8], F32, tag="att")
            pg = big[:, 0:8, :]
            # attention PSUM scratch shares the step tile: pq 8:10 | pw 10:14
            pq = big[:, 8:10, :]
            pw = big[:, 10:14, :]
            psc = big[:, 14, 0:4]
            pz = big[:, 15, 0:4]
            # q^T [DH, 8]
            for mt in range(2):
                for kt in range(2):
                    nc.tensor.matmul(pq[:, mt, :],
                                     lhsT=wh[:, kt, mt * 128:(mt + 1) * 128],
                                     rhs=h_prev[:, kt, :],
                                     start=(kt == 0), stop=(kt == 1))
            # h-dependent gate matmuls (independent of attention)
            for mt in range(4):
                for kt in range(2):
                    nc.tensor.matmul(pg[:, mt, :],
                                     lhsT=whhd[:, kt, mt * 128:(mt + 1) * 128],
                                     rhs=h_prev[:, kt, :],
                                     start=(kt == 0), stop=False,
                                     skip_group_check=True)
            for j in range(2):
                for kt in range(2):
                    nc.tensor.matmul(pg[:, 4 + j, :],
                                     lhsT=whhd[:, kt, (4 + j) * 128:(5 + j) * 128],
                                     rhs=h_prev[:, kt, :],
                                     start=(kt == 0), stop=(kt == 1),
                                     skip_group_check=True)
            q_bf = stp.tile([128, 2, 8], F32, tag="q_bf")
            nc.vector.tensor_copy(out=q_bf[:], in_=pq)
            pump(3)
            # energy = tanh(enc_proj + q): per-(kt,b) adds with q as the
            # per-partition scalar (packed bf16 SBUF operands -> fast DVE)
            energy = stp.tile([128, 2, 512], BF16, tag="energy")
            for h in range(4):
                sl = slice(h * 128, (h + 1) * 128)
                for b in (2 * h, 2 * h + 1):
                    for kt in range(2):
                        nc.vector.tensor_scalar(
                            out=energy[:, kt, b * 64:(b + 1) * 64],
                            in0=enc_proj[:, kt, b * 64:(b + 1) * 64],
                            scalar1=q_bf[:, kt, b:b + 1], scalar2=None,
                            op0=OP.add)
                nc.scalar.activation(out=energy[:, :, sl], in_=energy[:, :, sl],
                                     func=AF.Tanh)
            # scores -> psc [(b,s)-part, j]
            for j in range(4):
                for kt in range(2):
                    nc.tensor.matmul(
                        psc[:, j:j + 1],
                        lhsT=energy[:, kt, j * 128:(j + 1) * 128],
                        rhs=v_sb[:, kt:kt + 1], start=(kt == 0), stop=(kt == 1),
                        skip_group_check=True)
            pump(1)
            exp_f = stp.tile([128, 4], F32, tag="exp_f")
            nc.scalar.activation(out=exp_f[:], in_=psc, func=AF.Exp)
            nc.tensor.matmul(pz, lhsT=blk[:], rhs=exp_f[:], start=True, stop=True)
            rcp = stp.tile([128, 4], F32, tag="rcp")
            nc.vector.reciprocal(out=rcp[:], in_=pz)
            nc.vector.tensor_tensor(out=a_eo[0:64, :, 0], in0=exp_f[0:64, :],
                                    in1=rcp[0:64, :], op=OP.mult)
            nc.vector.tensor_tensor(out=a_eo[64:128, :, 1], in0=exp_f[64:128, :],
                                    in1=rcp[64:128, :], op=OP.mult)
            pump(1)
            # weighted^T [2EH, 8]
            for bp in range(4):
                for et in range(4):
                    nc.tensor.matmul(
                        pw[:, et, 2 * bp:2 * bp + 2],
                        lhsT=enc_pack[:, bp, et * 128:(et + 1) * 128],
                        rhs=a_eo[:, bp, :], start=True, stop=True)
            wdst = xcatT[:, 2:6, t * 8:(t + 1) * 8]
            nc.vector.tensor_copy(out=wdst, in_=pw)
            # w+emb gate matmuls: rhs = xcatT rows 2:7 (w | emb), 5 k-tiles
            xw = xcatT[:, 2:7, t * 8:(t + 1) * 8]
            for mt in range(4):
                for kt in range(5):
                    nc.tensor.matmul(pg[:, mt, :],
                                     lhsT=wihw[:, kt, mt * 128:(mt + 1) * 128],
                                     rhs=xw[:, kt, :],
                                     start=False, stop=(kt == 4),
                                     skip_group_check=True)
            for j in range(2):
                for kt in range(5):
                    nc.tensor.matmul(pg[:, 6 + j, :],
                                     lhsT=wihw[:, kt, (4 + j) * 128:(5 + j) * 128],
                                     rhs=xw[:, kt, :],
                                     start=(kt == 0), stop=(kt == 4),
                                     skip_group_check=True)
            # gates (sigmoid via tanh(x/2); whhd n-rows prescaled by 0.5)
            th = stp.tile([128, 4, 8], BF16, tag="d_th")
            nc.scalar.activation(out=th[:], in_=pg[:, 0:4, :], func=AF.Tanh,
                                 scale=0.5)
            t_n = stp.tile([128, 2, 8], BF16, tag="d_n")
            nc.vector.scalar_tensor_tensor(out=t_n[:], in0=th[:, 0:2, :],
                                           scalar=1.0, in1=pg[:, 4:6, :],
                                           op0=OP.add, op1=OP.mult)
            nc.vector.scalar_tensor_tensor(out=t_n[:], in0=t_n[:], scalar=1.0,
                                           in1=pg[:, 6:8, :], op0=OP.mult,
                                           op1=OP.add)
            n_t = stp.tile([128, 2, 8], BF16, tag="d_tanh")
            nc.scalar.activation(out=n_t[:], in_=t_n[:], func=AF.Tanh)
            d_t = stp.tile([128, 2, 8], BF16, tag="d_d")
            nc.vector.tensor_tensor(out=d_t[:], in0=h_prev, in1=n_t[:],
                                    op=OP.subtract)
            nc.vector.scalar_tensor_tensor(out=d_t[:], in0=th[:, 2:4, :],
                                           scalar=1.0, in1=d_t[:], op0=OP.add,
                                           op1=OP.mult)
            nc.vector.scalar_tensor_tensor(
                out=xcatT[:, 0:2, t * 8:(t + 1) * 8], in0=d_t[:],
                scalar=0.5, in1=n_t[:], op0=OP.mult, op1=OP.add)
            pump(3)

        gather_by_tf = {tf: g for g, (tf, _, _) in enumerate(GATHERS)}
        ready_by_t = {}
        tail_units = []
        for g, (tf, t0s, nst) in enumerate(GATHERS):
            mts = (nst * HB + 127) // 128
            units = [(g, mt, ns) for mt in range(mts) for ns in range(NCH)]
            lag = 8 if nst >= 8 else 7
            if tf + lag <= TD - 1:
                ready_by_t.setdefault(tf + lag, []).extend(units)
            else:
                tail_units.extend(units)

        fetch_by_t = {}
        for g, (tf, _, _) in enumerate(GATHERS):
            if tf + 6 <= TD - 1:
                fetch_by_t[tf + 6] = g
        fetched = set()
        for t in range(TD):
            cur_t[0] = t
            if t in ready_by_t:
                fc_queue.extend(ready_by_t[t])
            dec_step(t)
            if t in fetch_by_t:
                emit_fetch(fetch_by_t[t])
                fetched.add(fetch_by_t[t])
            if t in gather_by_tf:
                emit_gather(gather_by_tf[t])
        cur_t[0] = TD
        no_pool_steps.add(TD)
        for g in range(len(GATHERS)):
            if g not in fetched:
                emit_fetch(g)
        for item in fc_queue + tail_units:
            emit_fc_unit(*item)
        fcg_ctx.__exit__(None, None, None)
        fcps_ctx.__exit__(None, None, None)
        psA_ctx.__exit__(None, None, None)

    nc.compile()
    return nc


def _prep_inputs(inputs):
    """Host-side layout prep shared across cores. Returns (shared, per_core)."""
    f = {k: np.asarray(v) for k, v in inputs.items()}
    bf = lambda a: np.ascontiguousarray(a, dtype=np.float32).astype(bfnp)
    tr = lambda a: bf(np.asarray(a, np.float32).T)

    def half_n(whh):
        w = np.asarray(whh, np.float32).copy()
        w[2 * w.shape[0] // 3:, :] *= 0.5    # prescale n-gate rows
        return tr(w)

    def pk(a):
        a = np.asarray(a, bfnp)
        ko = a.shape[0] // 128
        return a.reshape(ko, 128, a.shape[1]).transpose(1, 0, 2).reshape(128, -1)

    blob = np.concatenate([
        pk(half_n(f["enc_Whh_f"])), pk(half_n(f["enc_Whh_b"])),
        pk(tr(f["enc_fc_W"])),
        pk(tr(f["attn_W"][:, :DH])), pk(tr(f["attn_W"][:, DH:])),
        pk(tr(np.concatenate([f["dec_Wih"][:, E:],
                              f["dec_Wih"][:, :E]], axis=1))),
        pk(half_n(f["dec_Whh"])),
        pk(tr(f["enc_Wih_f"])), pk(tr(f["enc_Wih_b"])),
        pk(tr(f["dec_Wih"][:, :E])),
        bf(f["attn_v"][0].reshape(2, 128).T),
        np.eye(128, dtype=bfnp),
    ], axis=1)
    assert blob.shape == (128, NBLOB), blob.shape

    shared = dict(
        enc_emb=bf(f["enc_emb"]),
        dec_emb=bf(f["dec_emb"]),
        wblob=np.ascontiguousarray(blob),
        blkones=np.kron(np.eye(2, dtype=np.float32), np.ones((64, 64), np.float32)),
    )

    src = np.asarray(f["src"])
    trg = np.asarray(f["trg"])
    fcwt_full = tr(f["fc_W"])                                     # [896, 32000]
    per_core = []
    for c in range(NCORES):
        cols = slice(c * BL, (c + 1) * BL)
        si = src[:, cols].astype(np.int32).reshape(-1)            # s-major, 512
        ti = trg[:TD, cols].astype(np.int32).reshape(-1)          # t-major, 504
        ti = np.concatenate([ti, np.zeros(8, np.int32)])
        tok = np.concatenate([si.reshape(4, 128), ti.reshape(4, 128)]).T  # [128, 8]
        vsh = c % NGRP
        per_core.append(dict(
            tok_idx=np.ascontiguousarray(tok),
            fcw_t=np.ascontiguousarray(fcwt_full[:, vsh * VS:(vsh + 1) * VS])))
    return shared, per_core


def kernel(**inputs):
    if "nc" not in _CACHE:
        _CACHE["nc"] = _build_program()
    nc = _CACHE["nc"]

    shared, per_core = _prep_inputs(inputs)
    in_maps = [{**shared, **pc} for pc in per_core]

    res = run_bass_kernel_spmd(nc, in_maps, core_ids=list(range(NCORES)))
    _CACHE["last_result"] = res

    out = np.zeros((T, B, V), np.float32)
    for c in range(NCORES):
        g, vsh = c // NGRP, c % NGRP
        arr = np.asarray(res.results[c]["out"], dtype=np.float32)
        out[1:, g * HB:(g + 1) * HB, vsh * VS:(vsh + 1) * VS] = \
            arr.reshape(TD, HB, VS)
    return out
